# revision 24
# baseline (speedup 1.0000x reference)
"""Trainium2 Bass kernel for nn_Attention_22874995818839.

Model: BatchNorm1d -> grouped 1x1 conv QKV (groups=8) -> channel-shuffle
split_heads (d-outer/h-inner) with q/k swap -> 8-head attention over N=2048,
D=32 -> 1x1 output conv with bias.

Sharding over 8 cores: core c owns batch b = c//4 and attention heads
{2*(c%4), 2*(c%4)+1}. Each core computes BN stats (over both batches) and the
full fused QKV projection for its batch, attention for its two heads, and the
output projection restricted to its 64 attention channels (row-sharded). The
host sums the 4 partial outputs per batch (the "all-reduce").

Key kernel-level choices:
- The grouped conv + channel shuffle is folded into dense 256x256 weight
  matrices built on the host, so the QKV matmul writes Q^T / K^T straight
  into SBUF in head-major layout, replicated 4x along partitions. That
  replication feeds `tile_position` row-packed K=32 score matmuls (4
  concurrent matmuls in the 128x128 PE array).
- The BatchNorm affine is folded into the QKV weights on device:
  W' = s_c * W rows, plus rank-1 bias corrections (tqk for Q/K as a
  per-partition add on the projection output; for V the correction commutes
  through softmax-averaging and becomes an output-projection bias `be`).
  So x feeds the matmuls directly as float32r - no normalization pass.
- All matmuls run in float32r: full PE rate at free-dim >= 256, near-fp32
  precision.
- Scores are computed transposed: S[j, i] = sum_d K[j,d] Q[i,d], so softmax's
  exp runs on ScalarE straight out of PSUM and no transposes are needed
  anywhere. exp is applied without max subtraction (logits are tiny).
- The softmax denominator l_i = sum_j exp(S[j,i]) falls out of the PV matmul
  for free via a ones-column appended to V (output row 32).
- BN: var -> rstd via exp(-0.5*ln(var+eps)), and sum(x^2) via ACT Square with
  accum_out, so the whole kernel uses one ACT table set
  (natural_log_exp_and_others) and the Square pass runs while ACT would
  otherwise idle during the prologue.

Pipeline tuning (measured on HW via direct R2-vs-R1 rep differencing; the
ScalarE exp stream at ~1364ns per [128,1024] ACTIVATE is the wall):
- Scores PSUM is a 3-deep ring (ps_s bufs=3) with all other small PSUM
  tiles (QKV chunks, norm broadcast, epilogue) allocated from the same
  ring, so the PE can run score groups further ahead of the exp stream and
  jitter in the PE->ACT handoff never gaps ACT. Alternative structures
  that lower the ACT instruction count but shallow the buffering (global
  [128,1536]/[128,1024] alternating exp segments) measured 25-40% slower
  on HW despite the lighter ACT workload - ring depth wins.
- BN statistics for both 128-channel chunks run as one batched [128,2]
  vector chain instead of two scalar chains, shortening the prologue.
- The 16 V-projection chunks are split 8/8 between block 0 and block 1,
  each emitted just ahead of its first PV use, so block 0's PE slots feed
  the first exp sooner.
- q4/k4/vx live in a double-buffered pool so back-to-back invocations of
  the body (pipelined calls / repeated-body timing programs) overlap: the
  next invocation's QKV projection can run under the previous one's late
  attention blocks instead of waiting for its final score matmul.
"""

import numpy as np

import concourse.bass as bass
import concourse.mybir as mybir
import concourse.tile as tile

B, C, N, H, D = 2, 256, 2048, 8, 32
EPS = 1e-5
SCALE = float(D) ** -0.5
F32 = mybir.dt.float32
F32R = mybir.dt.float32r
BF16 = mybir.dt.bfloat16
ALU = mybir.AluOpType
ACTF = mybir.ActivationFunctionType

CT = 2              # channel tiles of 128 (C = 256)
NIC, ICW = 4, 512   # query chunks
JBS, JBW = 16, 128  # key blocks
NGR = 4             # groups of 4 row-packed key blocks

_PROGRAM = None


def r32(ap):
    return ap.bitcast(F32R)


def _build_program(nreps=1):
    nc = bass.Bass("TRN2", target_bir_lowering=False, debug=False,
                   num_devices=8)
    x = nc.declare_dram_parameter("x_ord", [B, C, N], F32R, isOutput=False)
    wqa = nc.declare_dram_parameter("wqa", [C, 2 * 128], F32, isOutput=False)
    wka = nc.declare_dram_parameter("wka", [C, 2 * 128], F32, isOutput=False)
    wva = nc.declare_dram_parameter("wva", [C, 64], F32, isOutput=False)
    wot = nc.declare_dram_parameter("wot", [64, C], F32R, isOutput=False)
    gam = nc.declare_dram_parameter("gam", [C, 1], F32, isOutput=False)
    bet = nc.declare_dram_parameter("bet", [C, 1], F32, isOutput=False)
    bo4 = nc.declare_dram_parameter("bo4", [C, 1], F32, isOutput=False)
    vones = nc.declare_dram_parameter("vones", [128, 2 * JBS], F32R,
                                      isOutput=False)
    y = nc.declare_dram_parameter("y", [C, N], F32, isOutput=True)

    with tile.TileContext(nc) as tc:
        with (
            tc.tile_pool(name="big", bufs=1) as big,
            tc.tile_pool(name="big2", bufs=2) as big2,
            tc.tile_pool(name="scr", bufs=2) as scrp,
            tc.tile_pool(name="pp", bufs=12) as pp,
            tc.tile_pool(name="outp", bufs=2) as outp,
            tc.tile_pool(name="small", bufs=1) as small,
            tc.tile_pool(name="ps_s", bufs=3, space="PSUM") as ps_s,
            tc.tile_pool(name="ps_u", bufs=2, space="PSUM") as ps_u,
        ):
            for _rep in range(nreps):
                # ---------------- x DMA (chunked) ----------------
                xts = {}
                for ct in range(CT):
                    for bb in range(B):
                        t = big.tile([128, N], F32R, name=f"xt_{ct}_{bb}",
                                     tag=f"xt_{ct}_{bb}")
                        xts[(ct, bb)] = t
                        nc.sync.dma_start(t[:],
                                          x[bb, 128 * ct:128 * (ct + 1), :])

                # ---------------- weight / small input DMAs ----------------
                wq_sb, wk_sb, wv_sb = [], [], []
                bo4_sb = []
                gamb = small.tile([128, 2], F32, name="gamb", tag="gamb")
                betb = small.tile([128, 2], F32, name="betb", tag="betb")
                for ct in range(CT):
                    wqt = big.tile([128, 256], F32, name=f"wq_sb{ct}", tag=f"wq_sb{ct}")
                    nc.sync.dma_start(wqt[:], wqa[128 * ct:128 * (ct + 1), :])
                    wq_sb.append(wqt)
                    wkt = big.tile([128, 256], F32, name=f"wk_sb{ct}", tag=f"wk_sb{ct}")
                    nc.sync.dma_start(wkt[:], wka[128 * ct:128 * (ct + 1), :])
                    wk_sb.append(wkt)
                    wvt = big.tile([128, 64], F32, name=f"wv_sb{ct}", tag=f"wv_sb{ct}")
                    nc.sync.dma_start(wvt[:], wva[128 * ct:128 * (ct + 1), :])
                    wv_sb.append(wvt)
                    nc.sync.dma_start(gamb[:, ct:ct + 1],
                                      gam[128 * ct:128 * (ct + 1), :])
                    nc.sync.dma_start(betb[:, ct:ct + 1],
                                      bet[128 * ct:128 * (ct + 1), :])
                    t = small.tile([128, 1], F32, name=f"bo4_sb{ct}",
                                   tag=f"bo4_sb{ct}")
                    nc.sync.dma_start(t[:], bo4[128 * ct:128 * (ct + 1), :])
                    bo4_sb.append(t)
                wot_sb = big.tile([64, 256], F32R, name="wot_sb", tag="wot_sb")
                nc.sync.dma_start(wot_sb[:], wot[:, :])
                wotf = big.tile([64, 256], F32, name="wotf", tag="wotf")
                nc.sync.dma_start(wotf[:], wot[:, :].bitcast(F32))
                ones_sb = small.tile([1, 32], F32, name="ones_sb", tag="ones_sb")
                nc.vector.memset(ones_sb[:], 1.0)

                # ---------------- BN statistics ----------------
                # sum(x) on DVE, sum(x^2) on ACT (Square + accum_out, same table
                # set as Exp/Ln), both chunked to pipeline behind the x DMA.
                sp4 = small.tile([128, 4], F32, name="sp4", tag="sp4")
                qp4 = small.tile([128, 4], F32, name="qp4", tag="qp4")
                for ct in range(CT):
                    for bb in range(B):
                        ch = xts[(ct, bb)][:]
                        col = 2 * ct + bb
                        nc.vector.reduce_sum(sp4[:, col:col + 1], ch,
                                             axis=mybir.AxisListType.X)
                        scr = scrp.tile([128, N], BF16, name="scr", tag="scr")
                        nc.scalar.activation(scr[:], ch, ACTF.Square,
                                             accum_out=qp4[:, col:col + 1])
                # both chunks' stats in one [128,2] chain (col = ct)
                spr = sp4[:].rearrange("p (c b) -> p c b", b=2)
                qpr = qp4[:].rearrange("p (c b) -> p c b", b=2)
                mean2 = small.tile([128, 2], F32, name="mean2", tag="mean2")
                nc.vector.tensor_add(mean2[:], spr[:, :, 0], spr[:, :, 1])
                nc.vector.tensor_scalar_mul(mean2[:], mean2[:], 1.0 / (B * N))
                msq2 = small.tile([128, 2], F32, name="msq2", tag="msq2")
                nc.vector.tensor_add(msq2[:], qpr[:, :, 0], qpr[:, :, 1])
                nc.vector.tensor_scalar_mul(msq2[:], msq2[:], 1.0 / (B * N))
                var2 = small.tile([128, 2], F32, name="var2", tag="var2")
                nc.vector.tensor_mul(var2[:], mean2[:], mean2[:])
                nc.vector.tensor_sub(var2[:], msq2[:], var2[:])
                nc.vector.tensor_scalar_add(var2[:], var2[:], EPS)
                # rstd = exp(-0.5 * ln(var + eps)); Ln and Exp share one ACT set
                lnv2 = small.tile([128, 2], F32, name="lnv2", tag="lnv2")
                nc.scalar.activation(lnv2[:], var2[:], ACTF.Ln)
                sc2 = small.tile([128, 2], F32, name="sc2", tag="sc2")
                nc.scalar.activation(sc2[:], lnv2[:], ACTF.Exp, scale=-0.5)
                nc.vector.tensor_mul(sc2[:], sc2[:], gamb[:])
                t2b = small.tile([128, 2], F32, name="t2b", tag="t2b")
                nc.vector.tensor_mul(t2b[:], mean2[:], sc2[:])
                nc.vector.tensor_sub(t2b[:], betb[:], t2b[:])
                s_ct = [sc2[:, ct:ct + 1] for ct in range(CT)]
                t_ct = [t2b[:, ct:ct + 1] for ct in range(CT)]

                # -------- fold BN affine into weights + bias corrections --------
                # scaled weights (f32, used as f32r by the matmuls)
                wq2, wk2, wv2 = [], [], []
                for ct in range(CT):
                    for src, lst, nm in ((wq_sb, wq2, "wq2"), (wk_sb, wk2, "wk2"),
                                         (wv_sb, wv2, "wv2")):
                        t2 = big.tile([128, src[ct].shape[1]], F32R,
                                      name=f"{nm}_{ct}", tag=f"{nm}_{ct}")
                        nc.vector.tensor_scalar_mul(t2[:], src[ct][:], s_ct[ct])
                        lst.append(t2)
                # tqk[col] = sum_c t_c * W[c, col]: bias of the attention-Q
                # projection (wk). The K-side projection bias shifts every
                # logit of a query by the same constant, which softmax
                # cancels, so k4 needs no bias. The V-side bias commutes
                # through the softmax average into the output bias (be).
                tqk_sb = {}
                for hl in range(2):
                    tps = ps_s.tile([128, 1], F32, name="tps", tag="sc")
                    for ct in range(CT):
                        nc.tensor.matmul(tps[:],
                                         wq_sb[ct][:, 128 * hl:128 * (hl + 1)],
                                         t_ct[ct], start=(ct == 0),
                                         stop=(ct == CT - 1))
                    tsb = small.tile([128, 1], F32, name=f"tqk_q{hl}",
                                     tag=f"tqk_q{hl}")
                    nc.vector.tensor_copy(tsb[:], tps[:])
                    tqk_sb[hl] = tsb

                # ---------------- QKV + attention, software-pipelined ------
                # The engines execute their streams in order, so emission
                # order IS the schedule. ScalarE (exp) is the bottleneck;
                # everything else is woven into the stalls of the score
                # matmuls so ACT never starves: block k's scores interleave
                # with block k-1's PV matmuls, and the V / QKV chunks fill
                # block 0's slack.
                q4 = [None, None]
                k4 = [None, None]
                for hl in range(2):
                    q4[hl] = big2.tile([128, N], F32R, name=f"q4_{hl}",
                                       tag=f"q4_{hl}")
                    k4[hl] = big2.tile([128, N], F32R, name=f"k4_{hl}",
                                       tag=f"k4_{hl}")

                def emit_qk_chunk(hl, icn):
                    for (w2, dst, bias) in ((wq2, q4[hl], tqk_sb[hl]),
                                            (wk2, k4[hl], None)):
                        ps = ps_s.tile([128, 512], F32, name="mmps",
                                       tag="sc")
                        for ct in range(CT):
                            nc.tensor.matmul(
                                ps[:], w2[ct][:, 128 * hl:128 * (hl + 1)],
                                xts[(ct, 0)][:, 512 * icn:512 * (icn + 1)],
                                start=(ct == 0), stop=(ct == CT - 1))
                        sl = dst[:, 512 * icn:512 * (icn + 1)]
                        if bias is None:
                            nc.vector.tensor_copy(sl, ps[:])
                        else:
                            nc.vector.tensor_scalar_add(sl, ps[:], bias[:])

                # V: [n-part, d] layout per key block, with a ones column
                # (row 32 of the PV output = softmax denominator).
                vx = big2.tile([128, 2 * JBS * 33], F32R, name="vx", tag="vx")
                vx4 = vx.rearrange("p (h j w) -> p h j w", h=2, w=33)
                nc.sync.dma_start(
                    vx4[:, :, :, 32:33],
                    vones.ap().rearrange("p (h j) -> p h j", h=2)[:, :, :, None])
                ones_r = small.tile([1, 32], F32R, name="ones_r", tag="ones_r")
                nc.sync.dma_start(ones_r[:], vones[0:1, 0:32])

                def emit_v_chunk(nb):
                    ps = ps_s.tile([128, 64], F32, name="mmps", tag="sc")
                    for ct in range(CT):
                        nc.tensor.matmul(
                            ps[:], xts[(ct, 0)][:, 128 * nb:128 * (nb + 1)],
                            wv2[ct][:], start=(ct == 0), stop=(ct == CT - 1))
                    nc.vector.tensor_copy(
                        vx4[:, :, nb, 0:32],
                        ps.rearrange("p (h w) -> p h w", h=2)[:, :, :])

                blocks = [(ic, hl) for ic in range(NIC) for hl in range(2)]
                u_ps = {}
                att_t = {}
                pgs = {}

                def emit_scores_half(k, j):
                    ic, hl = blocks[k]
                    sps = ps_s.tile([128, 1024], F32, name="sps", tag="sc")
                    for rl_ in range(2):
                        jb = 2 * j + rl_
                        r = jb % 4
                        nc.tensor.matmul(
                            sps[:, 512 * rl_:512 * (rl_ + 1)],
                            k4[hl][32 * r:32 * (r + 1),
                                   128 * jb:128 * (jb + 1)],
                            q4[hl][32 * r:32 * (r + 1),
                                   512 * ic:512 * (ic + 1)],
                            start=True, stop=True, tile_position=(32 * r, 0))
                    pg = pp.tile([128, 1024], F32R, name="pg", tag="pg")
                    nc.scalar.activation(pg[:], sps[:], ACTF.Exp, scale=SCALE)
                    pgs[k].append(pg)

                def emit_pv_half(k, j):
                    ic, hl = blocks[k]
                    pg = pgs[k][j]
                    for rl_ in range(2):
                        jb = 2 * j + rl_
                        nc.tensor.matmul(
                            u_ps[k][:],
                            vx[:, (hl * JBS + jb) * 33:(hl * JBS + jb) * 33 + 33],
                            pg[:, 512 * rl_:512 * (rl_ + 1)],
                            start=(jb == 0), stop=(jb == JBS - 1),
                            skip_group_check=True)

                def emit_norm(k):
                    ic, hl = blocks[k]
                    ups = u_ps[k]
                    if ic not in att_t:
                        att_t[ic] = outp.tile([64, 512], F32R, name="att",
                                              tag="att")
                    rl = outp.tile([1, 512], F32R, name="rl", tag=f"rl{hl}")
                    with nc.allow_low_precision("softmax scale in f32r"):
                        nc.vector.reciprocal(rl[:], ups[32:33, :])
                    rlb = ps_s.tile([32, 512], F32, name="rlb", tag="sc")
                    nc.tensor.matmul(rlb[:], ones_r[:], rl[:],
                                     start=True, stop=True)
                    # DVE can read only one PSUM operand: stage rlb in SBUF
                    rls = outp.tile([32, 512], F32R, name="rls",
                                    tag=f"rls{hl}")
                    nc.vector.tensor_copy(rls[:], rlb[:])
                    nc.vector.tensor_mul(att_t[ic][32 * hl:32 * (hl + 1), :],
                                         ups[0:32, :], rls[:])

                def emit_epi(ic):
                    for ot in range(2):
                        yps = ps_s.tile([128, 512], F32, name="yps",
                                        tag="sc")
                        nc.tensor.matmul(yps[:],
                                         wot_sb[:, 128 * ot:128 * (ot + 1)],
                                         att_t[ic][:], start=True, stop=True)
                        ysb = outp.tile([128, 512], F32, name="ysb", tag="ysb")
                        nc.vector.tensor_scalar(
                            out=ysb[:], in0=yps[:], scalar1=bo4_sb[ot][:],
                            scalar2=be_sb[ot][:], op0=ALU.add, op1=ALU.add)
                        nc.sync.dma_start(
                            y[128 * ot:128 * (ot + 1),
                              512 * ic:512 * (ic + 1)], ysb[:])

                # block 0: scores paced by exp; QKV-h0 column chunks arrive
                # just before the score group that needs them; V fills slack.
                emit_qk_chunk(0, 0)
                u_ps[0] = ps_u.tile([33, 512], F32, name="ups", tag="u")
                pgs[0] = []
                for g in range(4):
                    if g >= 1:
                        emit_qk_chunk(0, g)
                    emit_scores_half(0, 2 * g)
                    emit_scores_half(0, 2 * g + 1)
                    for v in range(2):
                        emit_v_chunk(2 * g + v)

                # tv / be bias corrections (tiny; first needed at epi(0))
                tvps = ps_s.tile([64, 1], F32, name="tvps", tag="sc")
                for ct in range(CT):
                    nc.tensor.matmul(tvps[:], wv_sb[ct][:], t_ct[ct],
                                     start=(ct == 0), stop=(ct == CT - 1))
                tv_sb = small.tile([64, 1], F32, name="tv_sb", tag="tv_sb")
                nc.vector.tensor_copy(tv_sb[:], tvps[:])
                be_sb = []
                for ot in range(2):
                    bps = ps_s.tile([128, 1], F32, name="bps", tag="sc")
                    nc.tensor.matmul(bps[:], wotf[:, 128 * ot:128 * (ot + 1)],
                                     tv_sb[:], start=True, stop=True)
                    bsb = small.tile([128, 1], F32, name=f"be_sb{ot}",
                                     tag=f"be_sb{ot}")
                    nc.vector.tensor_copy(bsb[:], bps[:])
                    be_sb.append(bsb)

                for icn in range(4):
                    emit_qk_chunk(1, icn)

                # steady state: scores(k) interleaved with PV(k-1); the
                # last block also overlaps its own PV into its score slots
                # (lagged by 2 so the exp is ready) to shorten the tail.
                last = len(blocks) - 1
                for k in range(1, len(blocks)):
                    u_ps[k] = ps_u.tile([33, 512], F32, name="ups", tag="u")
                    pgs[k] = []
                    for g in range(4):
                        emit_scores_half(k, 2 * g)
                        emit_scores_half(k, 2 * g + 1)
                        if k == 1:
                            emit_v_chunk(8 + 2 * g)
                            emit_v_chunk(8 + 2 * g + 1)
                        emit_pv_half(k - 1, 2 * g)
                        emit_pv_half(k - 1, 2 * g + 1)
                        if k == last and g >= 1:
                            emit_pv_half(last, 2 * g - 2)
                            emit_pv_half(last, 2 * g - 1)
                    emit_norm(k - 1)
                    if (k - 1) % 2 == 1:
                        emit_epi((k - 1) // 2)
                for j in range(6, 8):
                    emit_pv_half(last, j)

                emit_norm(last)
                emit_epi(NIC - 1)
    return nc


def _get_program():
    global _PROGRAM
    if _PROGRAM is None:
        nc = _build_program()
        # Split multi-sem waits (TRN2 ISA allows one sync wait per
        # instruction); the rest of Bacc.compile() is skipped - its register
        # passes break the preamble registers under this runtime path.
        import bass_rust as _br
        _br.move_matmul_waits_to_ldweights(nc.m)
        _br.generate_event_semaphores(nc)
        _PROGRAM = nc
    return _PROGRAM


def _build_core_inputs(core, x, gamma, beta, wk, wq, wv, wo, bo):
    """Per-core numpy input map (pure layout work, no math)."""
    b = core // 4
    h0 = 2 * (core % 4)

    x_ord = np.ascontiguousarray(np.stack([x[b], x[1 - b]]).astype(np.float32))

    # split_heads channel map: attention head h, dim d2 <- conv channel d2*8+h
    def build_qk(w):
        W = np.zeros((C, 2 * 128), np.float32)
        for hl in range(2):
            h = h0 + hl
            for r in range(4):
                for d2 in range(D):
                    cref = d2 * 8 + h
                    g, dd = cref // 32, cref % 32
                    W[g * 32:(g + 1) * 32, hl * 128 + r * 32 + d2] = \
                        w[g * 32 + dd, :]
        return W

    # q/k swap: attention-Q comes from the wk projection, attention-K from wq
    wqa = build_qk(wk)
    wka = build_qk(wq)

    wva = np.zeros((C, 64), np.float32)
    for hl in range(2):
        h = h0 + hl
        for d2 in range(D):
            cref = d2 * 8 + h
            g, dd = cref // 32, cref % 32
            wva[g * 32:(g + 1) * 32, hl * 32 + d2] = wv[g * 32 + dd, :]


    wot = np.ascontiguousarray(wo[:, h0 * 32:(h0 + 2) * 32].T).astype(np.float32)

    return {
        "x_ord": x_ord,
        "vones": np.ones((128, 2 * JBS), np.float32),
        "wqa": wqa,
        "wka": wka,
        "wva": wva,
        "wot": wot,
        "gam": gamma.reshape(C, 1).astype(np.float32),
        "bet": beta.reshape(C, 1).astype(np.float32),
        "bo4": (bo / 4.0).reshape(C, 1).astype(np.float32),
    }


def kernel(x, gamma, beta, wk, wq, wv, wo, bo, _want_trace=False):
    x = np.asarray(x, np.float32)
    gamma = np.asarray(gamma, np.float32)
    beta = np.asarray(beta, np.float32)
    wk = np.asarray(wk, np.float32)
    wq = np.asarray(wq, np.float32)
    wv = np.asarray(wv, np.float32)
    wo = np.asarray(wo, np.float32)
    bo = np.asarray(bo, np.float32)

    from concourse.bass_utils import run_bass_kernel_spmd

    nc = _get_program()
    in_maps = [_build_core_inputs(c, x, gamma, beta, wk, wq, wv, wo, bo)
               for c in range(8)]
    res = run_bass_kernel_spmd(nc, in_maps, list(range(8)),
                               trace=_want_trace)

    out = np.zeros((B, C, N), np.float32)
    for c in range(8):
        out[c // 4] += res.results[c]["y"]
    if _want_trace:
        return out, res
    return out



# revision 26
# speedup vs baseline: 1.1582x; 1.1582x over previous
"""Trainium2 Bass kernel for nn_Attention_22874995818839.

Model: BatchNorm1d -> grouped 1x1 conv QKV (groups=8) -> channel-shuffle
split_heads (d-outer/h-inner) with q/k swap -> 8-head attention over N=2048,
D=32 -> 1x1 output conv with bias.

Sharding over 8 cores: core c owns batch b = c//4 and attention heads
{2*(c%4), 2*(c%4)+1}. Each core computes BN stats (over both batches) and the
full fused QKV projection for its batch, attention for its two heads, and the
output projection restricted to its 64 attention channels (row-sharded). The
host sums the 4 partial outputs per batch (the "all-reduce").

Key kernel-level choices:
- The grouped conv + channel shuffle is folded into dense 256x256 weight
  matrices built on the host, so the QKV matmul writes Q^T / K^T straight
  into SBUF in head-major layout, replicated 4x along partitions. That
  replication feeds `tile_position` row-packed K=32 score matmuls (4
  concurrent matmuls in the 128x128 PE array).
- The BatchNorm affine is folded into the QKV weights on device:
  W' = s_c * W rows, plus rank-1 bias corrections (tqk for Q/K as a
  per-partition add on the projection output; for V the correction commutes
  through softmax-averaging and becomes an output-projection bias `be`).
  So x feeds the matmuls directly as float32r - no normalization pass.
- All matmuls run in float32r: full PE rate at free-dim >= 256, near-fp32
  precision.
- Scores are computed transposed: S[j, i] = sum_d K[j,d] Q[i,d], so softmax's
  exp runs on ScalarE straight out of PSUM and no transposes are needed
  anywhere. exp is applied without max subtraction (logits are tiny).
- The softmax denominator l_i = sum_j exp(S[j,i]) falls out of the PV matmul
  for free via a ones-column appended to V (output row 32).
- BN: var -> rstd via exp(-0.5*ln(var+eps)), and sum(x^2) via ACT Square with
  accum_out, so the whole kernel uses one ACT table set
  (natural_log_exp_and_others) and the Square pass runs while ACT would
  otherwise idle during the prologue.

Pipeline tuning (measured on HW via direct R2-vs-R1 rep differencing; the
ScalarE exp stream at ~1364ns per [128,1024] ACTIVATE is the wall):
- Scores PSUM is a 3-deep ring (ps_s bufs=3) with all other small PSUM
  tiles (QKV chunks, norm broadcast, epilogue) allocated from the same
  ring, so the PE can run score groups further ahead of the exp stream and
  jitter in the PE->ACT handoff never gaps ACT. Alternative structures
  that lower the ACT instruction count but shallow the buffering (global
  [128,1536]/[128,1024] alternating exp segments) measured 25-40% slower
  on HW despite the lighter ACT workload - ring depth wins.
- BN statistics for both 128-channel chunks run as one batched [128,2]
  vector chain instead of two scalar chains, shortening the prologue.
- The 16 V-projection chunks are split 8/8 between block 0 and block 1,
  each emitted just ahead of its first PV use, so block 0's PE slots feed
  the first exp sooner.
- q4/k4/vx live in a double-buffered pool so back-to-back invocations of
  the body (pipelined calls / repeated-body timing programs) overlap: the
  next invocation's QKV projection can run under the previous one's late
  attention blocks instead of waiting for its final score matmul.
"""

import ml_dtypes
import numpy as np

import concourse.bass as bass
import concourse.mybir as mybir
import concourse.tile as tile

B, C, N, H, D = 2, 256, 2048, 8, 32
EPS = 1e-5
SCALE = float(D) ** -0.5
F32 = mybir.dt.float32
F32R = mybir.dt.float32r
BF16 = mybir.dt.bfloat16
ALU = mybir.AluOpType
ACTF = mybir.ActivationFunctionType

CT = 2              # channel tiles of 128 (C = 256)
NIC, ICW = 4, 512   # query chunks
JBS, JBW = 16, 128  # key blocks
NGR = 4             # groups of 4 row-packed key blocks

_PROGRAM = None


def r32(ap):
    return ap.bitcast(F32R)


def _build_program(nreps=1):
    nc = bass.Bass("TRN2", target_bir_lowering=False, debug=False,
                   num_devices=8)
    x = nc.declare_dram_parameter("x_ord", [B, C, N], F32R, isOutput=False)
    wqa = nc.declare_dram_parameter("wqa", [C, 2 * 128], F32, isOutput=False)
    wka = nc.declare_dram_parameter("wka", [C, 2 * 128], F32, isOutput=False)
    wva = nc.declare_dram_parameter("wva", [C, 64], F32, isOutput=False)
    wot = nc.declare_dram_parameter("wot", [64, C], F32R, isOutput=False)
    gam = nc.declare_dram_parameter("gam", [C, 1], F32, isOutput=False)
    bet = nc.declare_dram_parameter("bet", [C, 1], F32, isOutput=False)
    bo4 = nc.declare_dram_parameter("bo4", [C, 1], F32, isOutput=False)
    vones = nc.declare_dram_parameter("vones", [128, 2 * JBS], BF16,
                                      isOutput=False)
    vonesf = nc.declare_dram_parameter("vonesf", [1, 32], F32R,
                                       isOutput=False)
    y = nc.declare_dram_parameter("y", [C, N], F32, isOutput=True)

    with tile.TileContext(nc) as tc:
        with (
            tc.tile_pool(name="big", bufs=1) as big,
            tc.tile_pool(name="big2", bufs=2) as big2,
            tc.tile_pool(name="scr", bufs=2) as scrp,
            tc.tile_pool(name="pp", bufs=24) as pp,
            tc.tile_pool(name="outp", bufs=2) as outp,
            tc.tile_pool(name="small", bufs=1) as small,
            tc.tile_pool(name="ps_s", bufs=3, space="PSUM") as ps_s,
            tc.tile_pool(name="ps_u", bufs=2, space="PSUM") as ps_u,
        ):
            for _rep in range(nreps):
                # ---------------- x DMA (chunked) ----------------
                xts = {}
                for ct in range(CT):
                    for bb in range(B):
                        t = big.tile([128, N], F32R, name=f"xt_{ct}_{bb}",
                                     tag=f"xt_{ct}_{bb}")
                        xts[(ct, bb)] = t
                        nc.sync.dma_start(t[:],
                                          x[bb, 128 * ct:128 * (ct + 1), :])

                # ---------------- weight / small input DMAs ----------------
                wq_sb, wk_sb, wv_sb = [], [], []
                bo4_sb = []
                gamb = small.tile([128, 2], F32, name="gamb", tag="gamb")
                betb = small.tile([128, 2], F32, name="betb", tag="betb")
                for ct in range(CT):
                    wqt = big.tile([128, 256], F32, name=f"wq_sb{ct}", tag=f"wq_sb{ct}")
                    nc.sync.dma_start(wqt[:], wqa[128 * ct:128 * (ct + 1), :])
                    wq_sb.append(wqt)
                    wkt = big.tile([128, 256], F32, name=f"wk_sb{ct}", tag=f"wk_sb{ct}")
                    nc.sync.dma_start(wkt[:], wka[128 * ct:128 * (ct + 1), :])
                    wk_sb.append(wkt)
                    wvt = big.tile([128, 64], F32, name=f"wv_sb{ct}", tag=f"wv_sb{ct}")
                    nc.sync.dma_start(wvt[:], wva[128 * ct:128 * (ct + 1), :])
                    wv_sb.append(wvt)
                    nc.sync.dma_start(gamb[:, ct:ct + 1],
                                      gam[128 * ct:128 * (ct + 1), :])
                    nc.sync.dma_start(betb[:, ct:ct + 1],
                                      bet[128 * ct:128 * (ct + 1), :])
                    t = big2.tile([128, 1], F32, name=f"bo4_sb{ct}",
                                  tag=f"bo4_sb{ct}")
                    nc.sync.dma_start(t[:], bo4[128 * ct:128 * (ct + 1), :])
                    bo4_sb.append(t)
                wot_sb = big2.tile([64, 256], F32R, name="wot_sb", tag="wot_sb")
                nc.sync.dma_start(wot_sb[:], wot[:, :])
                wotf = big.tile([64, 256], F32, name="wotf", tag="wotf")
                nc.sync.dma_start(wotf[:], wot[:, :].bitcast(F32))
                ones_sb = small.tile([1, 32], F32, name="ones_sb", tag="ones_sb")
                nc.vector.memset(ones_sb[:], 1.0)

                # ---------------- BN statistics ----------------
                # sum(x) on DVE, sum(x^2) on ACT (Square + accum_out, same table
                # set as Exp/Ln), both chunked to pipeline behind the x DMA.
                sp4 = small.tile([128, 4], F32, name="sp4", tag="sp4")
                qp4 = small.tile([128, 4], F32, name="qp4", tag="qp4")
                for ct in range(CT):
                    for bb in range(B):
                        ch = xts[(ct, bb)][:]
                        col = 2 * ct + bb
                        nc.vector.reduce_sum(sp4[:, col:col + 1], ch,
                                             axis=mybir.AxisListType.X)
                        scr = scrp.tile([128, N], BF16, name="scr", tag="scr")
                        nc.scalar.activation(scr[:], ch, ACTF.Square,
                                             accum_out=qp4[:, col:col + 1])
                # both chunks' stats in one [128,2] chain (col = ct)
                spr = sp4[:].rearrange("p (c b) -> p c b", b=2)
                qpr = qp4[:].rearrange("p (c b) -> p c b", b=2)
                mean2 = small.tile([128, 2], F32, name="mean2", tag="mean2")
                nc.vector.tensor_add(mean2[:], spr[:, :, 0], spr[:, :, 1])
                nc.vector.tensor_scalar_mul(mean2[:], mean2[:], 1.0 / (B * N))
                msq2 = small.tile([128, 2], F32, name="msq2", tag="msq2")
                nc.vector.tensor_add(msq2[:], qpr[:, :, 0], qpr[:, :, 1])
                nc.vector.tensor_scalar_mul(msq2[:], msq2[:], 1.0 / (B * N))
                var2 = small.tile([128, 2], F32, name="var2", tag="var2")
                nc.vector.tensor_mul(var2[:], mean2[:], mean2[:])
                nc.vector.tensor_sub(var2[:], msq2[:], var2[:])
                nc.vector.tensor_scalar_add(var2[:], var2[:], EPS)
                # rstd = exp(-0.5 * ln(var + eps)); Ln and Exp share one ACT set
                lnv2 = small.tile([128, 2], F32, name="lnv2", tag="lnv2")
                nc.scalar.activation(lnv2[:], var2[:], ACTF.Ln)
                sc2 = small.tile([128, 2], F32, name="sc2", tag="sc2")
                nc.scalar.activation(sc2[:], lnv2[:], ACTF.Exp, scale=-0.5)
                nc.vector.tensor_mul(sc2[:], sc2[:], gamb[:])
                t2b = small.tile([128, 2], F32, name="t2b", tag="t2b")
                nc.vector.tensor_mul(t2b[:], mean2[:], sc2[:])
                nc.vector.tensor_sub(t2b[:], betb[:], t2b[:])
                s_ct = [sc2[:, ct:ct + 1] for ct in range(CT)]
                t_ct = [t2b[:, ct:ct + 1] for ct in range(CT)]

                # -------- fold BN affine into weights + bias corrections --------
                # scaled weights (f32, used as f32r by the matmuls)
                wq2, wk2, wv2 = [], [], []
                for ct in range(CT):
                    for src, lst, nm in ((wq_sb, wq2, "wq2"), (wk_sb, wk2, "wk2"),
                                         (wv_sb, wv2, "wv2")):
                        t2 = big.tile([128, src[ct].shape[1]], F32R,
                                      name=f"{nm}_{ct}", tag=f"{nm}_{ct}")
                        nc.vector.tensor_scalar_mul(t2[:], src[ct][:], s_ct[ct])
                        lst.append(t2)
                # tqk[col] = sum_c t_c * W[c, col]: bias of the attention-Q
                # projection (wk). The K-side projection bias shifts every
                # logit of a query by the same constant, which softmax
                # cancels, so k4 needs no bias. The V-side bias commutes
                # through the softmax average into the output bias (be).
                tqk_sb = {}
                for hl in range(2):
                    tps = ps_s.tile([128, 1], F32, name="tps", tag="sc")
                    for ct in range(CT):
                        nc.tensor.matmul(tps[:],
                                         wq_sb[ct][:, 128 * hl:128 * (hl + 1)],
                                         t_ct[ct], start=(ct == 0),
                                         stop=(ct == CT - 1))
                    tsb = small.tile([128, 1], F32, name=f"tqk_q{hl}",
                                     tag=f"tqk_q{hl}")
                    nc.vector.tensor_copy(tsb[:], tps[:])
                    tqk_sb[hl] = tsb

                # ---------------- QKV + attention, software-pipelined ------
                # The engines execute their streams in order, so emission
                # order IS the schedule. ScalarE (exp) is the bottleneck;
                # everything else is woven into the stalls of the score
                # matmuls so ACT never starves: block k's scores interleave
                # with block k-1's PV matmuls, and the V / QKV chunks fill
                # block 0's slack.
                q4 = [None, None]
                k4 = [None, None]
                for hl in range(2):
                    q4[hl] = big2.tile([128, N], F32R, name=f"q4_{hl}",
                                       tag=f"q4_{hl}")
                    k4[hl] = big2.tile([128, N], F32R, name=f"k4_{hl}",
                                       tag=f"k4_{hl}")

                def emit_qk_chunk(hl, icn):
                    for (w2, dst, bias) in ((wq2, q4[hl], tqk_sb[hl]),
                                            (wk2, k4[hl], None)):
                        ps = ps_s.tile([128, 512], F32, name="mmps",
                                       tag="sc")
                        for ct in range(CT):
                            nc.tensor.matmul(
                                ps[:], w2[ct][:, 128 * hl:128 * (hl + 1)],
                                xts[(ct, 0)][:, 512 * icn:512 * (icn + 1)],
                                start=(ct == 0), stop=(ct == CT - 1))
                        sl = dst[:, 512 * icn:512 * (icn + 1)]
                        if bias is None:
                            nc.vector.tensor_copy(sl, ps[:])
                        else:
                            nc.vector.tensor_scalar_add(sl, ps[:], bias[:])

                # V: [n-part, d] layout per key block, with a ones column
                # (row 32 of the PV output = softmax denominator).
                vx = big2.tile([128, 2 * JBS * 33], BF16, name="vx", tag="vx")
                vx4 = vx.rearrange("p (h j w) -> p h j w", h=2, w=33)
                nc.sync.dma_start(
                    vx4[:, :, :, 32:33],
                    vones.ap().rearrange("p (h j) -> p h j", h=2)[:, :, :, None])
                ones_r = big2.tile([1, 32], F32R, name="ones_r",
                                   tag="ones_r")
                nc.sync.dma_start(ones_r[:], vonesf[:, :])

                def emit_v_chunk(nb):
                    ps = ps_s.tile([128, 64], F32, name="mmps", tag="sc")
                    for ct in range(CT):
                        nc.tensor.matmul(
                            ps[:], xts[(ct, 0)][:, 128 * nb:128 * (nb + 1)],
                            wv2[ct][:], start=(ct == 0), stop=(ct == CT - 1))
                    nc.vector.tensor_copy(
                        vx4[:, :, nb, 0:32],
                        ps.rearrange("p (h w) -> p h w", h=2)[:, :, :])

                blocks = [(ic, hl) for ic in range(NIC) for hl in range(2)]
                u_ps = {}
                att_t = {}
                pgs = {}

                def emit_scores_half(k, j):
                    ic, hl = blocks[k]
                    sps = ps_s.tile([128, 1024], F32, name="sps", tag="sc")
                    for rl_ in range(2):
                        jb = 2 * j + rl_
                        r = jb % 4
                        nc.tensor.matmul(
                            sps[:, 512 * rl_:512 * (rl_ + 1)],
                            k4[hl][32 * r:32 * (r + 1),
                                   128 * jb:128 * (jb + 1)],
                            q4[hl][32 * r:32 * (r + 1),
                                   512 * ic:512 * (ic + 1)],
                            start=True, stop=True, tile_position=(32 * r, 0))
                    pg = pp.tile([128, 1024], BF16, name="pg", tag="pg")
                    nc.scalar.activation(pg[:], sps[:], ACTF.Exp, scale=SCALE)
                    pgs[k].append(pg)

                def emit_pv_half(k, j):
                    ic, hl = blocks[k]
                    pg = pgs[k][j]
                    for rl_ in range(2):
                        jb = 2 * j + rl_
                        nc.tensor.matmul(
                            u_ps[k][:],
                            vx[:, (hl * JBS + jb) * 33:(hl * JBS + jb) * 33 + 33],
                            pg[:, 512 * rl_:512 * (rl_ + 1)],
                            start=(jb == 0), stop=(jb == JBS - 1),
                            skip_group_check=True)

                def emit_norm(k):
                    ic, hl = blocks[k]
                    ups = u_ps[k]
                    if ic not in att_t:
                        att_t[ic] = outp.tile([64, 512], F32R, name="att",
                                              tag="att")
                    rl = outp.tile([1, 512], F32R, name="rl", tag=f"rl{hl}")
                    with nc.allow_low_precision("softmax scale in f32r"):
                        nc.vector.reciprocal(rl[:], ups[32:33, :])
                    rlb = ps_s.tile([32, 512], F32, name="rlb", tag="sc")
                    nc.tensor.matmul(rlb[:], ones_r[:], rl[:],
                                     start=True, stop=True)
                    # DVE can read only one PSUM operand: stage rlb in SBUF
                    rls = outp.tile([32, 512], F32R, name="rls",
                                    tag=f"rls{hl}")
                    nc.vector.tensor_copy(rls[:], rlb[:])
                    nc.vector.tensor_mul(att_t[ic][32 * hl:32 * (hl + 1), :],
                                         ups[0:32, :], rls[:])

                def emit_epi(ic):
                    for ot in range(2):
                        yps = ps_s.tile([128, 512], F32, name="yps",
                                        tag="sc")
                        nc.tensor.matmul(yps[:],
                                         wot_sb[:, 128 * ot:128 * (ot + 1)],
                                         att_t[ic][:], start=True, stop=True)
                        ysb = outp.tile([128, 512], F32, name="ysb", tag="ysb")
                        nc.vector.tensor_scalar(
                            out=ysb[:], in0=yps[:], scalar1=bo4_sb[ot][:],
                            scalar2=be_sb[ot][:], op0=ALU.add, op1=ALU.add)
                        nc.sync.dma_start(
                            y[128 * ot:128 * (ot + 1),
                              512 * ic:512 * (ic + 1)], ysb[:])

                # block 0: scores paced by exp; QKV-h0 column chunks arrive
                # just before the score group that needs them; V fills slack.
                emit_qk_chunk(0, 0)
                u_ps[0] = ps_u.tile([33, 512], F32, name="ups", tag="u")
                pgs[0] = []
                for g in range(4):
                    if g >= 1:
                        emit_qk_chunk(0, g)
                    emit_scores_half(0, 2 * g)
                    emit_scores_half(0, 2 * g + 1)
                    for v in range(2):
                        emit_v_chunk(2 * g + v)

                # tv / be bias corrections (tiny; first needed at epi(0))
                tvps = ps_s.tile([64, 1], F32, name="tvps", tag="sc")
                for ct in range(CT):
                    nc.tensor.matmul(tvps[:], wv_sb[ct][:], t_ct[ct],
                                     start=(ct == 0), stop=(ct == CT - 1))
                tv_sb = small.tile([64, 1], F32, name="tv_sb", tag="tv_sb")
                nc.vector.tensor_copy(tv_sb[:], tvps[:])
                be_sb = []
                for ot in range(2):
                    bps = ps_s.tile([128, 1], F32, name="bps", tag="sc")
                    nc.tensor.matmul(bps[:], wotf[:, 128 * ot:128 * (ot + 1)],
                                     tv_sb[:], start=True, stop=True)
                    bsb = big2.tile([128, 1], F32, name=f"be_sb{ot}",
                                    tag=f"be_sb{ot}")
                    nc.vector.tensor_copy(bsb[:], bps[:])
                    be_sb.append(bsb)

                for icn in range(4):
                    emit_qk_chunk(1, icn)

                # steady state: scores(k) interleaved with PV(k-1); the
                # last block also overlaps its own PV into its score slots
                # (lagged by 2 so the exp is ready) to shorten the tail.
                last = len(blocks) - 1
                for k in range(1, len(blocks)):
                    u_ps[k] = ps_u.tile([33, 512], F32, name="ups", tag="u")
                    pgs[k] = []
                    for g in range(4):
                        emit_scores_half(k, 2 * g)
                        emit_scores_half(k, 2 * g + 1)
                        if k == 1:
                            emit_v_chunk(8 + 2 * g)
                            emit_v_chunk(8 + 2 * g + 1)
                        emit_pv_half(k - 1, 2 * g)
                        emit_pv_half(k - 1, 2 * g + 1)
                        if k == last and g >= 1:
                            emit_pv_half(last, 2 * g - 2)
                            emit_pv_half(last, 2 * g - 1)
                    emit_norm(k - 1)
                    if (k - 1) % 2 == 1:
                        emit_epi((k - 1) // 2)
                for j in range(6, 8):
                    emit_pv_half(last, j)

                emit_norm(last)
                emit_epi(NIC - 1)
    return nc


def _get_program():
    global _PROGRAM
    if _PROGRAM is None:
        nc = _build_program()
        # Split multi-sem waits (TRN2 ISA allows one sync wait per
        # instruction); the rest of Bacc.compile() is skipped - its register
        # passes break the preamble registers under this runtime path.
        import bass_rust as _br
        _br.move_matmul_waits_to_ldweights(nc.m)
        _br.generate_event_semaphores(nc)
        _PROGRAM = nc
    return _PROGRAM


def _build_core_inputs(core, x, gamma, beta, wk, wq, wv, wo, bo):
    """Per-core numpy input map (pure layout work, no math)."""
    b = core // 4
    h0 = 2 * (core % 4)

    x_ord = np.ascontiguousarray(np.stack([x[b], x[1 - b]]).astype(np.float32))

    # split_heads channel map: attention head h, dim d2 <- conv channel d2*8+h
    def build_qk(w):
        W = np.zeros((C, 2 * 128), np.float32)
        for hl in range(2):
            h = h0 + hl
            for r in range(4):
                for d2 in range(D):
                    cref = d2 * 8 + h
                    g, dd = cref // 32, cref % 32
                    W[g * 32:(g + 1) * 32, hl * 128 + r * 32 + d2] = \
                        w[g * 32 + dd, :]
        return W

    # q/k swap: attention-Q comes from the wk projection, attention-K from wq
    wqa = build_qk(wk)
    wka = build_qk(wq)

    wva = np.zeros((C, 64), np.float32)
    for hl in range(2):
        h = h0 + hl
        for d2 in range(D):
            cref = d2 * 8 + h
            g, dd = cref // 32, cref % 32
            wva[g * 32:(g + 1) * 32, hl * 32 + d2] = wv[g * 32 + dd, :]


    wot = np.ascontiguousarray(wo[:, h0 * 32:(h0 + 2) * 32].T).astype(np.float32)

    return {
        "x_ord": x_ord,
        "vones": np.ones((128, 2 * JBS), ml_dtypes.bfloat16),
        "vonesf": np.ones((1, 32), np.float32),
        "wqa": wqa,
        "wka": wka,
        "wva": wva,
        "wot": wot,
        "gam": gamma.reshape(C, 1).astype(np.float32),
        "bet": beta.reshape(C, 1).astype(np.float32),
        "bo4": (bo / 4.0).reshape(C, 1).astype(np.float32),
    }


def kernel(x, gamma, beta, wk, wq, wv, wo, bo, _want_trace=False):
    x = np.asarray(x, np.float32)
    gamma = np.asarray(gamma, np.float32)
    beta = np.asarray(beta, np.float32)
    wk = np.asarray(wk, np.float32)
    wq = np.asarray(wq, np.float32)
    wv = np.asarray(wv, np.float32)
    wo = np.asarray(wo, np.float32)
    bo = np.asarray(bo, np.float32)

    from concourse.bass_utils import run_bass_kernel_spmd

    nc = _get_program()
    in_maps = [_build_core_inputs(c, x, gamma, beta, wk, wq, wv, wo, bo)
               for c in range(8)]
    res = run_bass_kernel_spmd(nc, in_maps, list(range(8)),
                               trace=_want_trace)

    out = np.zeros((B, C, N), np.float32)
    for c in range(8):
        out[c // 4] += res.results[c]["y"]
    if _want_trace:
        return out, res
    return out

